# revision 17
# baseline (speedup 1.0000x reference)
"""Trainium2 Bass kernel for the ComplexMixture density-matrix problem.

Math (per batch b), with R = input_real[b] [S, D], I = input_imag[b] [S, D],
w = weight[b] [S]:
    out_r[b] = R^T diag(w) R + I^T diag(w) I      (symmetric)
    out_i[b] = I^T diag(w) R - R^T diag(w) I      (antisymmetric)
Contraction is over S, which maps directly onto the PE array's partition
(K) dimension -- no input transposes needed.

Kernel algorithm:
  * 3-multiplication (Karatsuba/Gauss) complex product.  Since w >= 0 we
    scale both sides by g = sqrt(w) (one fused scale+cast per operand):
        gr = g*R, gin = -g*I   (bf16)
        P1 = gr^T @ gr = R^T w R
        Q2 = gin^T @ gin = I^T w I
        P3 = (gr-gin)^T @ (gr+gin) = (R+I)^T w (R-I)
        out_r = P1 + Q2
        out_i = P3 - P1 + Q2
    3 big matmuls per batch instead of 4, and no separate cast pass.
  * Hermitian symmetry: only the upper-triangular 128-row strips of the
    outputs are computed on the PE (58% of the matmul work); the lower
    triangle is filled by PE-transposing the computed 128x128 tiles
    (negated for out_i).  Transposes are emitted one block late so they
    never head-of-line-block the next block's matmuls in the PE queue.
  * bf16 operands, fp32 PSUM accumulation (bf16 matmul is 4x fp32 rate).

Sharding: data-parallel over batch B=16 across 8 NeuronCores (2 per core),
no collectives.
"""

import sys

if "/opt/trn_rl_repo" not in sys.path:
    sys.path.insert(0, "/opt/trn_rl_repo")

import numpy as np

# Problem constants (hardcoded per harness contract)
B, S, D = 16, 1024, 768
N_CORES = 8
BPC = B // N_CORES  # batches per core
P = 128
KT = S // P   # 8 k-tiles along S
JT = D // P   # 6 column tiles of 128 along D


def _strip_blocks(m):
    """Upper-triangular strip m: computed column range [m*128, D) split
    into PSUM-bank-sized blocks (<=512 fp32)."""
    c0 = m * P
    width = D - c0
    blocks = []
    while width > 0:
        w = min(512, width)
        if width - w == 128 and w == 512:
            w = 384  # keep remainder >= 256 where possible
        blocks.append((c0, w))
        c0 += w
        width -= w
    return blocks


_PROGRAM = None


def _build_program():
    import concourse.mybir as mybir
    import concourse.tile as tile
    from concourse import bacc
    from concourse.masks import make_identity

    f32 = mybir.dt.float32
    bf16 = mybir.dt.bfloat16

    nc = bacc.Bacc("TRN2", target_bir_lowering=False, debug=False,
                   num_devices=N_CORES)

    r_dram = nc.dram_tensor("input_real", [BPC, S, D], f32, kind="ExternalInput")
    i_dram = nc.dram_tensor("input_imag", [BPC, S, D], f32, kind="ExternalInput")
    w_dram = nc.dram_tensor("weight", [BPC, S], f32, kind="ExternalInput")
    or_dram = nc.dram_tensor("out_r", [BPC, D, D], f32, kind="ExternalOutput")
    oi_dram = nc.dram_tensor("out_i", [BPC, D, D], f32, kind="ExternalOutput")

    # DRAM views with S split into (k, p)
    r_kp = r_dram.ap().rearrange("b (k p) d -> b p k d", p=P)
    i_kp = i_dram.ap().rearrange("b (k p) d -> b p k d", p=P)

    with tile.TileContext(nc) as tc:
        with (
            tc.tile_pool(name="const", bufs=1) as const_pool,
            tc.tile_pool(name="stage", bufs=5) as stage,
            tc.tile_pool(name="big", bufs=2) as big,
            tc.tile_pool(name="psum", bufs=2, space="PSUM") as psum,
            tc.tile_pool(name="psum_t", bufs=2, space="PSUM") as psum_t,
            tc.tile_pool(name="outp", bufs=3) as outp,
            tc.tile_pool(name="mirr", bufs=2) as mirr,
        ):
            # weight: [BPC, S] -> SBUF [128, BPC*KT]; column b*KT+k holds
            # w[b, k*128:(k+1)*128]
            w_sb = const_pool.tile([P, BPC * KT], f32)
            nc.sync.dma_start(
                w_sb[:], w_dram.ap().rearrange("b (k p) -> p (b k)", p=P, k=KT)
            )
            # g = sqrt(w), gneg = -sqrt(w)
            g_sb = const_pool.tile([P, BPC * KT], f32)
            gn_sb = const_pool.tile([P, BPC * KT], f32)
            nc.scalar.activation(g_sb[:], w_sb[:],
                                 mybir.ActivationFunctionType.Sqrt)
            nc.scalar.mul(gn_sb[:], g_sb[:], -1.0)
            ident = const_pool.tile([P, P], f32)
            make_identity(nc, ident[:])

            KC = 2  # k-tiles per input DMA chunk

            def emit_prep(b, ops):
                """loads + elementwise prep for one batch; returns operand set"""
                gr = big.tile([P, KT, D], bf16, tag="gr")    # g*R
                gi = big.tile([P, KT, D], bf16, tag="gi")    # -g*I
                ga = big.tile([P, KT, D], bf16, tag="ga")    # g*(R+I) = gr-gi
                gb = big.tile([P, KT, D], bf16, tag="gb")    # g*(R-I) = gr+gi
                stages = []
                for kc in range(KT // KC):
                    ks = slice(kc * KC, (kc + 1) * KC)
                    r32 = stage.tile([P, KC, D], f32, tag="r32")
                    i32 = stage.tile([P, KC, D], f32, tag="i32")
                    nc.sync.dma_start(r32[:], r_kp[b, :, ks, :])
                    nc.sync.dma_start(i32[:], i_kp[b, :, ks, :])
                    stages.append((r32, i32))
                for kc in range(KT // KC):
                    r32, i32 = stages[kc]
                    for dk in range(KC):
                        k = kc * KC + dk
                        gcol = g_sb[:, b * KT + k: b * KT + k + 1]
                        gncol = gn_sb[:, b * KT + k: b * KT + k + 1]
                        # fused scale+cast on DVE
                        nc.vector.tensor_scalar_mul(gr[:, k, :], r32[:, dk, :], gcol)
                        nc.vector.tensor_scalar_mul(gi[:, k, :], i32[:, dk, :], gncol)
                        nc.vector.tensor_sub(ga[:, k, :], gr[:, k, :], gi[:, k, :])
                        nc.gpsimd.tensor_add(gb[:, k, :], gr[:, k, :], gi[:, k, :])
                ops[b] = (gr, gi, ga, gb)

            pending = []  # deferred transpose/flush emitters

            def emit_pending():
                for fn in pending:
                    fn()
                pending.clear()

            def emit_groups(b, ops):
                gr, gi, ga, gb = ops[b]
                for m in range(JT):
                    ms = slice(m * P, (m + 1) * P)
                    nj = JT - 1 - m
                    if nj > 0:
                        mr_t = mirr.tile([P, nj, P], f32, tag="mr")
                        mi_t = mirr.tile([P, nj, P], f32, tag="mi")
                    blocks = _strip_blocks(m)
                    for bi, (c0, W) in enumerate(blocks):
                        cs = slice(c0, c0 + W)
                        p1 = psum.tile([P, W], f32, tag="p1")
                        q2 = psum.tile([P, W], f32, tag="q2")
                        p3 = psum.tile([P, W], f32, tag="p3")
                        for k in range(KT):
                            nc.tensor.matmul(p1[:], gr[:, k, ms], gr[:, k, cs],
                                             start=(k == 0), stop=(k == KT - 1))
                        for k in range(KT):
                            nc.tensor.matmul(q2[:], gi[:, k, ms], gi[:, k, cs],
                                             start=(k == 0), stop=(k == KT - 1))
                        for k in range(KT):
                            nc.tensor.matmul(p3[:], ga[:, k, ms], gb[:, k, cs],
                                             start=(k == 0), stop=(k == KT - 1))

                        # combine (DVE reads at most one PSUM operand per op)
                        c1_t = outp.tile([P, W], f32, tag="c1_t")
                        or_t = outp.tile([P, W], f32, tag="or_t")
                        ti_t = outp.tile([P, W], f32, tag="ti_t")
                        oi_t = outp.tile([P, W], f32, tag="oi_t")
                        nc.scalar.copy(c1_t[:], p1[:])
                        nc.vector.tensor_add(or_t[:], c1_t[:], q2[:])
                        nc.vector.tensor_sub(ti_t[:], p3[:], c1_t[:])
                        nc.vector.tensor_add(oi_t[:], ti_t[:], q2[:])
                        nc.sync.dma_start(or_dram[b, ms, cs], or_t[:])
                        nc.sync.dma_start(oi_dram[b, ms, cs], oi_t[:])

                        # previous block's transposes land in the PE queue
                        # behind this block's matmuls (no head-of-line stall)
                        emit_pending()

                        def mk_transposes(m=m, c0=c0, W=W, or_t=or_t,
                                          oi_t=oi_t, mr_t=mr_t if nj else None,
                                          mi_t=mi_t if nj else None,
                                          last=(bi == len(blocks) - 1), b=b):
                            j0 = max(c0 // P, m + 1)
                            for j in range(j0, (c0 + W) // P):
                                off = j * P - c0
                                tr = psum_t.tile([P, P], f32, tag="tr")
                                nc.tensor.transpose(tr[:], or_t[:, off:off + P],
                                                    ident[:])
                                nc.scalar.copy(mr_t[:, j - m - 1, :], tr[:])
                                ti2 = psum_t.tile([P, P], f32, tag="tr")
                                nc.tensor.transpose(ti2[:], oi_t[:, off:off + P],
                                                    ident[:])
                                nc.scalar.mul(mi_t[:, j - m - 1, :], ti2[:], -1.0)
                            if last and mr_t is not None:
                                rows = slice((m + 1) * P, D)
                                ms2 = slice(m * P, (m + 1) * P)
                                cview_r = or_dram[b, rows, ms2].rearrange(
                                    "(j p) r -> p j r", p=P)
                                cview_i = oi_dram[b, rows, ms2].rearrange(
                                    "(j p) r -> p j r", p=P)
                                nc.sync.dma_start(cview_r, mr_t[:])
                                nc.sync.dma_start(cview_i, mi_t[:])

                        pending.append(mk_transposes)
                emit_pending()

            ops = {}
            for b in range(BPC):
                emit_prep(b, ops)
            for b in range(BPC):
                emit_groups(b, ops)

    nc.compile()
    return nc


def _get_program():
    global _PROGRAM
    if _PROGRAM is None:
        _PROGRAM = _build_program()
    return _PROGRAM


def kernel(input_real, input_imag, weight, _spmd_kwargs=None):
    input_real = np.ascontiguousarray(input_real, dtype=np.float32)
    input_imag = np.ascontiguousarray(input_imag, dtype=np.float32)
    weight = np.ascontiguousarray(weight, dtype=np.float32)

    from concourse.bass_utils import run_bass_kernel_spmd

    nc = _get_program()
    in_maps = []
    for c in range(N_CORES):
        lo, hi = c * BPC, (c + 1) * BPC
        in_maps.append({
            "input_real": input_real[lo:hi],
            "input_imag": input_imag[lo:hi],
            "weight": weight[lo:hi],
        })
    res = run_bass_kernel_spmd(nc, in_maps, list(range(N_CORES)),
                               **(_spmd_kwargs or {}))
    out_r = np.concatenate([res.results[c]["out_r"] for c in range(N_CORES)], 0)
    out_i = np.concatenate([res.results[c]["out_i"] for c in range(N_CORES)], 0)
    kernel.last_results = res
    return (out_r, out_i)


# revision 19
# speedup vs baseline: 1.1261x; 1.1261x over previous
"""Trainium2 Bass kernel for the ComplexMixture density-matrix problem.

Math (per batch b), with R = input_real[b] [S, D], I = input_imag[b] [S, D],
w = weight[b] [S]:
    out_r[b] = R^T diag(w) R + I^T diag(w) I      (symmetric)
    out_i[b] = I^T diag(w) R - R^T diag(w) I      (antisymmetric)
Contraction is over S, which maps directly onto the PE array's partition
(K) dimension -- no input transposes needed.

Kernel algorithm:
  * 3-multiplication (Karatsuba/Gauss) complex product.  Since w >= 0 we
    scale both sides by g = sqrt(w) (one fused scale+cast per operand):
        gr = g*R, gin = -g*I   (bf16)
        P1 = gr^T @ gr = R^T w R
        Q2 = gin^T @ gin = I^T w I
        P3 = (gr-gin)^T @ (gr+gin) = (R+I)^T w (R-I)
        out_r = P1 + Q2
        out_i = P3 - P1 + Q2
    3 big matmuls per batch instead of 4, and no separate cast pass.
  * Hermitian symmetry: only the upper-triangular 128-row strips of the
    outputs are computed on the PE (58% of the matmul work); the lower
    triangle is filled by PE-transposing the computed 128x128 tiles
    (negated for out_i).  Transposes are emitted one block late so they
    never head-of-line-block the next block's matmuls in the PE queue.
  * bf16 operands, fp32 PSUM accumulation (bf16 matmul is 4x fp32 rate).

Sharding: data-parallel over batch B=16 across 8 NeuronCores (2 per core),
no collectives.
"""

import sys

if "/opt/trn_rl_repo" not in sys.path:
    sys.path.insert(0, "/opt/trn_rl_repo")

import numpy as np

# Problem constants (hardcoded per harness contract)
B, S, D = 16, 1024, 768
N_CORES = 8
BPC = B // N_CORES  # batches per core
P = 128
KT = S // P   # 8 k-tiles along S
JT = D // P   # 6 column tiles of 128 along D


def _strip_blocks(m):
    """Upper-triangular strip m: computed column range [m*128, D) split
    into PSUM-bank-sized blocks (<=512 fp32)."""
    c0 = m * P
    width = D - c0
    blocks = []
    while width > 0:
        w = min(512, width)
        if width - w == 128 and w == 512:
            w = 384  # keep remainder >= 256 where possible
        blocks.append((c0, w))
        c0 += w
        width -= w
    return blocks


_PROGRAM = None


def _build_program():
    import concourse.mybir as mybir
    import concourse.tile as tile
    from concourse import bacc
    from concourse.masks import make_identity

    f32 = mybir.dt.float32
    bf16 = mybir.dt.bfloat16

    nc = bacc.Bacc("TRN2", target_bir_lowering=False, debug=False,
                   num_devices=N_CORES)

    r_dram = nc.dram_tensor("input_real", [BPC, S, D], f32, kind="ExternalInput")
    i_dram = nc.dram_tensor("input_imag", [BPC, S, D], f32, kind="ExternalInput")
    w_dram = nc.dram_tensor("weight", [BPC, S], f32, kind="ExternalInput")
    or_dram = nc.dram_tensor("out_r", [BPC, D, D], f32, kind="ExternalOutput")
    oi_dram = nc.dram_tensor("out_i", [BPC, D, D], f32, kind="ExternalOutput")

    # DRAM views with S split into (k, p)
    r_kp = r_dram.ap().rearrange("b (k p) d -> b p k d", p=P)
    i_kp = i_dram.ap().rearrange("b (k p) d -> b p k d", p=P)

    with tile.TileContext(nc) as tc:
        with (
            tc.tile_pool(name="const", bufs=1) as const_pool,
            tc.tile_pool(name="stage", bufs=5) as stage,
            tc.tile_pool(name="big", bufs=2) as big,
            tc.tile_pool(name="psum", bufs=2, space="PSUM") as psum,
            tc.tile_pool(name="psum_t", bufs=2, space="PSUM") as psum_t,
            tc.tile_pool(name="outp", bufs=3) as outp,
            tc.tile_pool(name="mirr", bufs=2) as mirr,
        ):
            # weight: [BPC, S] -> SBUF [128, BPC*KT]; column b*KT+k holds
            # w[b, k*128:(k+1)*128]
            w_sb = const_pool.tile([P, BPC * KT], f32)
            nc.sync.dma_start(
                w_sb[:], w_dram.ap().rearrange("b (k p) -> p (b k)", p=P, k=KT)
            )
            # g = sqrt(w), gneg = -sqrt(w)
            g_sb = const_pool.tile([P, BPC * KT], f32)
            gn_sb = const_pool.tile([P, BPC * KT], f32)
            nc.scalar.activation(g_sb[:], w_sb[:],
                                 mybir.ActivationFunctionType.Sqrt)
            nc.scalar.mul(gn_sb[:], g_sb[:], -1.0)
            ident = const_pool.tile([P, P], f32)
            make_identity(nc, ident[:])

            KC = 2  # k-tiles per input DMA chunk

            def emit_prep(b, ops):
                """loads + elementwise prep for one batch; returns operand set"""
                gr = big.tile([P, KT, D], bf16, tag="gr")    # g*R
                gi = big.tile([P, KT, D], bf16, tag="gi")    # -g*I
                ga = big.tile([P, KT, D], bf16, tag="ga")    # g*(R+I) = gr-gi
                gb = big.tile([P, KT, D], bf16, tag="gb")    # g*(R-I) = gr+gi
                stages = []
                for kc in range(KT // KC):
                    ks = slice(kc * KC, (kc + 1) * KC)
                    r32 = stage.tile([P, KC, D], f32, tag="r32")
                    i32 = stage.tile([P, KC, D], f32, tag="i32")
                    nc.sync.dma_start(r32[:], r_kp[b, :, ks, :])
                    nc.sync.dma_start(i32[:], i_kp[b, :, ks, :])
                    stages.append((r32, i32))
                for kc in range(KT // KC):
                    r32, i32 = stages[kc]
                    for dk in range(KC):
                        k = kc * KC + dk
                        gcol = g_sb[:, b * KT + k: b * KT + k + 1]
                        gncol = gn_sb[:, b * KT + k: b * KT + k + 1]
                        # fused scale+cast: gr on DVE, gi on ACT (parallel)
                        nc.vector.tensor_scalar_mul(gr[:, k, :], r32[:, dk, :], gcol)
                        nc.scalar.mul(gi[:, k, :], i32[:, dk, :], gncol)
                        nc.vector.tensor_sub(ga[:, k, :], gr[:, k, :], gi[:, k, :])
                        nc.vector.tensor_add(gb[:, k, :], gr[:, k, :], gi[:, k, :])
                ops[b] = (gr, gi, ga, gb)

            pending = []  # deferred transpose/flush emitters

            def emit_pending():
                for fn in pending:
                    fn()
                pending.clear()

            def emit_groups(b, ops):
                gr, gi, ga, gb = ops[b]
                for m in range(JT):
                    ms = slice(m * P, (m + 1) * P)
                    nj = JT - 1 - m
                    if nj > 0:
                        mr_t = mirr.tile([P, nj, P], f32, tag="mr")
                        mi_t = mirr.tile([P, nj, P], f32, tag="mi")
                    blocks = _strip_blocks(m)
                    for bi, (c0, W) in enumerate(blocks):
                        cs = slice(c0, c0 + W)
                        p1 = psum.tile([P, W], f32, tag="p1")
                        q2 = psum.tile([P, W], f32, tag="q2")
                        p3 = psum.tile([P, W], f32, tag="p3")
                        for k in range(KT):
                            nc.tensor.matmul(p1[:], gr[:, k, ms], gr[:, k, cs],
                                             start=(k == 0), stop=(k == KT - 1))
                        for k in range(KT):
                            nc.tensor.matmul(q2[:], gi[:, k, ms], gi[:, k, cs],
                                             start=(k == 0), stop=(k == KT - 1))
                        for k in range(KT):
                            nc.tensor.matmul(p3[:], ga[:, k, ms], gb[:, k, cs],
                                             start=(k == 0), stop=(k == KT - 1))

                        # combine (DVE reads at most one PSUM operand per op)
                        c1_t = outp.tile([P, W], f32, tag="c1_t")
                        or_t = outp.tile([P, W], f32, tag="or_t")
                        ti_t = outp.tile([P, W], f32, tag="ti_t")
                        oi_t = outp.tile([P, W], f32, tag="oi_t")
                        nc.scalar.copy(c1_t[:], p1[:])
                        nc.vector.tensor_add(or_t[:], c1_t[:], q2[:])
                        nc.vector.tensor_sub(ti_t[:], p3[:], c1_t[:])
                        nc.vector.tensor_add(oi_t[:], ti_t[:], q2[:])
                        nc.sync.dma_start(or_dram[b, ms, cs], or_t[:])
                        nc.sync.dma_start(oi_dram[b, ms, cs], oi_t[:])

                        # previous block's transposes land in the PE queue
                        # behind this block's matmuls (no head-of-line stall)
                        emit_pending()

                        def mk_transposes(m=m, c0=c0, W=W, or_t=or_t,
                                          oi_t=oi_t, mr_t=mr_t if nj else None,
                                          mi_t=mi_t if nj else None,
                                          last=(bi == len(blocks) - 1), b=b):
                            j0 = max(c0 // P, m + 1)
                            for j in range(j0, (c0 + W) // P):
                                off = j * P - c0
                                tr = psum_t.tile([P, P], f32, tag="tr")
                                nc.tensor.transpose(tr[:], or_t[:, off:off + P],
                                                    ident[:])
                                nc.scalar.copy(mr_t[:, j - m - 1, :], tr[:])
                                ti2 = psum_t.tile([P, P], f32, tag="tr")
                                nc.tensor.transpose(ti2[:], oi_t[:, off:off + P],
                                                    ident[:])
                                nc.scalar.mul(mi_t[:, j - m - 1, :], ti2[:], -1.0)
                            if last and mr_t is not None:
                                rows = slice((m + 1) * P, D)
                                ms2 = slice(m * P, (m + 1) * P)
                                cview_r = or_dram[b, rows, ms2].rearrange(
                                    "(j p) r -> p j r", p=P)
                                cview_i = oi_dram[b, rows, ms2].rearrange(
                                    "(j p) r -> p j r", p=P)
                                nc.gpsimd.dma_start(cview_r, mr_t[:])
                                nc.gpsimd.dma_start(cview_i, mi_t[:])

                        pending.append(mk_transposes)
                emit_pending()

            ops = {}
            for b in range(BPC):
                emit_prep(b, ops)
            for b in range(BPC):
                emit_groups(b, ops)

    nc.compile()
    return nc


def _get_program():
    global _PROGRAM
    if _PROGRAM is None:
        _PROGRAM = _build_program()
    return _PROGRAM


def kernel(input_real, input_imag, weight, _spmd_kwargs=None):
    input_real = np.ascontiguousarray(input_real, dtype=np.float32)
    input_imag = np.ascontiguousarray(input_imag, dtype=np.float32)
    weight = np.ascontiguousarray(weight, dtype=np.float32)

    from concourse.bass_utils import run_bass_kernel_spmd

    nc = _get_program()
    in_maps = []
    for c in range(N_CORES):
        lo, hi = c * BPC, (c + 1) * BPC
        in_maps.append({
            "input_real": input_real[lo:hi],
            "input_imag": input_imag[lo:hi],
            "weight": weight[lo:hi],
        })
    res = run_bass_kernel_spmd(nc, in_maps, list(range(N_CORES)),
                               **(_spmd_kwargs or {}))
    out_r = np.concatenate([res.results[c]["out_r"] for c in range(N_CORES)], 0)
    out_i = np.concatenate([res.results[c]["out_i"] for c in range(N_CORES)], 0)
    kernel.last_results = res
    return (out_r, out_i)
